# revision 4
# baseline (speedup 1.0000x reference)
"""AnchorDiffNet fused attention kernel for 8 TRN2 NeuronCores.

Data-parallel over batch: B=8 samples -> 8 cores, no collectives.

Per core (one sample, C=128 channels, M=H*W=4096 positions):
  p0 = softmax(scale * ref^T cur), p1 = softmax(scale * cur^T cur)
  feats0 = cur @ p0^T, feats1 = cur @ p1^T
  y = lrelu(w1' @ [feats0; feats1; cur] + b1')   (BN folded into w1/b1)
  pred = w2 @ y + b2

Flash-style dataflow (nothing M x M ever touches HBM):
  - S^T chunks [n=128, m=512] = matmul(lhsT=cur[:, nchunk], rhs=mov[:, msuper])
    (both operands in natural (c, x) layout -> zero transposes)
  - E^T = Exp(scale * S^T) on ScalarE straight out of PSUM (no max subtraction
    needed: logits are ~N(0,1), |S|max ~ 6)
  - PV: out[m,129] = sum_n E^T[n,m-block]^T @ [curT | ones]; column 128 of the
    moving operand is all-ones, so the softmax denominator r[m] falls out of
    the same matmul for free.
  - normalize with 1/r (per-partition scalar), PE-transpose feats^T -> feats
  - head: three 128x128 stationary matmuls + leaky-relu + w2 row.
"""

import sys

sys.path.insert(0, "/opt/trn_rl_repo")

import numpy as np
import ml_dtypes

from concourse import bass, bacc, tile, mybir
from concourse.bass_utils import run_bass_kernel_spmd

B, C, H, W = 8, 128, 64, 64
M = H * W            # 4096
SUP = 512            # m-super width (moving free dim)
NSUP = M // SUP      # 8
CH = 128             # n-chunk width (contraction tile)
NCH = M // CH        # 32
SCALE = float(C) ** -0.5
F32 = mybir.dt.float32
BF16 = mybir.dt.bfloat16
BF = ml_dtypes.bfloat16
FX = mybir.ActivationFunctionType
OP = mybir.AluOpType


def build(debug=False):
    nc = bacc.Bacc("TRN2", target_bir_lowering=False, debug=debug, num_devices=8)

    ref_d = nc.dram_tensor("refm", (C, M), BF16, kind="ExternalInput")
    cur_d = nc.dram_tensor("curm", (C, M), BF16, kind="ExternalInput")
    ct1_d = nc.dram_tensor("curt1", (C, NCH * 129), BF16, kind="ExternalInput")
    w1t_d = nc.dram_tensor("w1t", (C, 3 * C), BF16, kind="ExternalInput")
    b1p_d = nc.dram_tensor("b1p", (C, 1), F32, kind="ExternalInput")
    w2t_d = nc.dram_tensor("w2t", (C, 1), BF16, kind="ExternalInput")
    b2s_d = nc.dram_tensor("b2s", (1, 1), F32, kind="ExternalInput")
    idn_d = nc.dram_tensor("idn", (C, C), BF16, kind="ExternalInput")
    out_d = nc.dram_tensor("out", (1, M), F32, kind="ExternalOutput")

    with tile.TileContext(nc) as tc:
        with (
            tc.tile_pool(name="const", bufs=1) as cp,
            tc.tile_pool(name="et", bufs=2) as etp,
            tc.tile_pool(name="work", bufs=2) as wp,
            tc.tile_pool(name="ftt", bufs=9) as ftp,
            tc.tile_pool(name="psS", bufs=2, space="PSUM") as psS,
            tc.tile_pool(name="ps2", bufs=4, space="PSUM") as ps2,
        ):
            ref_sb = cp.tile([C, M], BF16, tag="ref")
            cur_sb = cp.tile([C, M], BF16, tag="cur")
            ct1_sb = cp.tile([C, NCH, 129], BF16, tag="ct1")
            w1t_sb = cp.tile([C, 3 * C], BF16, tag="w1t")
            b1p_sb = cp.tile([C, 1], F32, tag="b1p")
            w2t_sb = cp.tile([C, 1], BF16, tag="w2t")
            b2s_sb = cp.tile([1, 1], F32, tag="b2s")
            idn_sb = cp.tile([C, C], BF16, tag="idn")
            nc.sync.dma_start(ref_sb[:], ref_d.ap())
            nc.sync.dma_start(cur_sb[:], cur_d.ap())
            nc.sync.dma_start(ct1_sb[:], ct1_d.ap().rearrange("c (k j) -> c k j", k=NCH))
            nc.sync.dma_start(w1t_sb[:], w1t_d.ap())
            nc.sync.dma_start(b1p_sb[:], b1p_d.ap())
            nc.sync.dma_start(w2t_sb[:], w2t_d.ap())
            nc.sync.dma_start(b2s_sb[:], b2s_d.ap())
            nc.sync.dma_start(idn_sb[:], idn_d.ap())

            for s in range(NSUP):
                ms = slice(s * SUP, (s + 1) * SUP)
                ftts = [[None] * 4, [None] * 4]
                for a in range(2):
                    mov = ref_sb if a == 0 else cur_sb
                    et = etp.tile([C, NCH, SUP], BF16, tag="et")
                    # S^T + exp, two n-chunks per PSUM tile
                    for j in range(NCH // 2):
                        ps = psS.tile([C, 2, SUP], F32, tag="s")
                        for d in range(2):
                            k = 2 * j + d
                            nc.tensor.matmul(
                                ps[:, d, :],
                                lhsT=cur_sb[:, k * CH : (k + 1) * CH],
                                rhs=mov[:, ms],
                                start=True,
                                stop=True,
                            )
                        nc.scalar.activation(
                            et[:, 2 * j : 2 * j + 2, :], ps[:], FX.Exp, scale=SCALE
                        )
                    # PV: k outer so PE consumption tracks ACT production
                    pvs = [
                        ps2.tile([C, 129], F32, tag="acc", name=f"pv{s}_{a}_{i}")
                        for i in range(4)
                    ]
                    for k in range(NCH):
                        for mb in range(4):
                            nc.tensor.matmul(
                                pvs[mb][:],
                                lhsT=et[:, k, mb * CH : (mb + 1) * CH],
                                rhs=ct1_sb[:, k, :],
                                start=(k == 0),
                                stop=(k == NCH - 1),
                            )
                    for mb in range(4):
                        rr = ftp.tile([C, 1], F32, tag="rr")
                        nc.vector.reciprocal_approx_fast(rr[:], pvs[mb][:, 128:129])
                        ftt = ftp.tile([C, C], BF16, tag=f"ftt{a}")
                        nc.vector.tensor_scalar(
                            ftt[:], pvs[mb][:, 0:128], rr[:], None, OP.mult
                        )
                        ftts[a][mb] = ftt
                # transposes (deferred so PE never waits on the DVE chain)
                feats = [None, None]
                for a in range(2):
                    f = wp.tile([C, SUP], BF16, tag=f"feats{a}")
                    for mb in range(4):
                        pt = ps2.tile([C, C], BF16, tag="acc")
                        nc.tensor.transpose(pt[:], ftts[a][mb][:], idn_sb[:])
                        nc.vector.tensor_copy(f[:, mb * CH : (mb + 1) * CH], pt[:])
                    feats[a] = f
                # head
                py = ps2.tile([C, SUP], F32, tag="acc")
                xs = [feats[0], feats[1], None]
                for kc in range(3):
                    rhs = cur_sb[:, ms] if kc == 2 else xs[kc][:]
                    nc.tensor.matmul(
                        py[:],
                        lhsT=w1t_sb[:, kc * C : (kc + 1) * C],
                        rhs=rhs,
                        start=(kc == 0),
                        stop=(kc == 2),
                    )
                yb = wp.tile([C, SUP], F32, tag="yb")
                nc.vector.tensor_scalar(yb[:], py[:], b1p_sb[:], None, OP.add)
                yt = wp.tile([C, SUP], F32, tag="yt")
                nc.vector.tensor_scalar(yt[:], yb[:], 0.01, None, OP.mult)
                ym = wp.tile([C, SUP], BF16, tag="ym")
                nc.vector.tensor_tensor(ym[:], yb[:], yt[:], OP.max)
                pw = ps2.tile([1, SUP], F32, tag="acc")
                nc.tensor.matmul(pw[:], lhsT=w2t_sb[:], rhs=ym[:], start=True, stop=True)
                ob = wp.tile([1, SUP], F32, tag="ob")
                nc.vector.tensor_scalar(ob[:], pw[:], b2s_sb[:], None, OP.add)
                nc.sync.dma_start(out_d.ap()[:, ms], ob[:])

    nc.compile()
    return nc


def prep_inputs(ref_feat, curr_feat, w1, b1, gamma, beta, running_mean, running_var, w2, b2):
    """Host-side prep: BN fold, scale fold, transposes, bf16 casts."""
    ref_feat = np.asarray(ref_feat, np.float32)
    curr_feat = np.asarray(curr_feat, np.float32)
    w1 = np.asarray(w1, np.float32)
    inv = np.asarray(gamma, np.float32) / np.sqrt(np.asarray(running_var, np.float32) + 1e-5)
    w1p = w1 * inv[:, None]
    b1p = (np.asarray(b1, np.float32) * inv + np.asarray(beta, np.float32)
           - np.asarray(running_mean, np.float32) * inv)
    w1t = np.ascontiguousarray(
        w1p.reshape(C, 3, C).transpose(2, 1, 0).reshape(C, 3 * C)
    ).astype(BF)
    w2t = np.ascontiguousarray(np.asarray(w2, np.float32).T).astype(BF)
    b2s = np.asarray(b2, np.float32).reshape(1, 1)
    b1p = b1p.reshape(C, 1)
    idn = np.eye(C, dtype=np.float32).astype(BF)

    in_maps = []
    for b in range(B):
        ref_m = ref_feat[b].reshape(C, M)
        cur_m = curr_feat[b].reshape(C, M)
        t = cur_m.reshape(C, NCH, CH).transpose(2, 1, 0)  # (n_in_chunk, k, c)
        ct1 = np.concatenate([t, np.ones((CH, NCH, 1), np.float32)], axis=2)
        in_maps.append({
            "refm": ref_m.astype(BF),
            "curm": cur_m.astype(BF),
            "curt1": np.ascontiguousarray(ct1.reshape(CH, NCH * 129)).astype(BF),
            "w1t": w1t,
            "b1p": b1p,
            "w2t": w2t,
            "b2s": b2s,
            "idn": idn,
        })
    return in_maps


_NC = None


def kernel(**inputs):
    global _NC
    if _NC is None:
        _NC = build(debug=False)
    in_maps = prep_inputs(**inputs)
    res = run_bass_kernel_spmd(_NC, in_maps, core_ids=list(range(B)))
    out = np.stack([np.asarray(res.results[i]["out"], np.float32).reshape(1, H, W)
                    for i in range(B)])
    return out
